# revision 5
# baseline (speedup 1.0000x reference)
"""Contextual-attention Trainium2 kernel (Bass/Tile), data-parallel over batch.

Math (per sequence b):
    Q = evo @ q_w.T + q_b                                  (L, 96)
    K = cat(evo, conv3(evo), conv5(evo)) @ k_w.T + k_b     (L, 96)
    V = plm @ v_w.T + v_b                                  (L, 96)
    P = softmax(Q K^T / sqrt(96), key-masked by seqlen)
    out = P @ V + V

Device-side reformulation (per core = one sequence):
  * The two convs + concat + K-projection fold into 5 shifted matmuls:
        K[l] = sum_{t=-2..2} evo[l+t] @ taps[t]  + bk      (host-folded weights)
  * Everything is computed transposed ([feature, L] layout) so the only
    contraction layouts needed are natural ones:
        QT = wqT.T @ evoT, KT = taps.T @ evoT(shifted), VT = wvT.T @ plmT
        ST[lk, lq] = KT_slice.T @ QT  -> exp via ACT with per-partition mask bias
        OT[0:96]   = sum_lk V1[lk].T @ ET[lk]   (V1 = [V | ones] natural layout,
        OT[96]     = softmax denominator         via on-chip PE transpose of VT)
  * Key tiles entirely beyond max(seqlen) are skipped at build time; the
    per-core mask bias (0 / -1e6) zeroes partially-valid tiles exactly
    (exp(-1e6 + s) underflows to 0.0f, matching the reference's where()+softmax).
  * Final divide by denominator, +V residual, and the (96, L) -> (L, 96)
    transpose happen on host (tiny O(L*96) work).
"""

import os
import numpy as np

import concourse.bacc as bacc
import concourse.bass as bass
import concourse.tile as tile
from concourse import mybir
from concourse._compat import get_trn_type
from concourse.bass_utils import run_bass_kernel_spmd

B, L = 8, 2048
Q_IN, V_IN, QK, VD = 512, 1024, 96, 96
P = 128
NORM = float(1.0 / np.sqrt(QK))
F32 = mybir.dt.float32

LAST_EXEC_TIME_NS = None
LAST_RESULTS = None

_program_cache = {}


def _fold_k_weights(k_w, k_b, cn3_w, cn3_b, cn5_w, cn5_b):
    """K[l] = sum_{t in -2..2} evo[l+t] @ taps[t+2] + bk  (zero-padded shifts)."""
    A_evo = k_w[:, :Q_IN]
    A3 = k_w[:, Q_IN : Q_IN + VD]
    A5 = k_w[:, Q_IN + VD :]
    taps = np.zeros((5, Q_IN, QK), np.float32)
    for j in range(3):  # conv3 tap j acts at offset t = j-1
        taps[j - 1 + 2] += np.einsum("oc,cd->do", A3, cn3_w[:, :, j]).astype(np.float32)
    for j in range(5):  # conv5 tap j acts at offset t = j-2
        taps[j - 2 + 2] += np.einsum("oc,cd->do", A5, cn5_w[:, :, j]).astype(np.float32)
    taps[2] += A_evo.T
    bk = (k_b + A3 @ cn3_b + A5 @ cn5_b).astype(np.float32)
    return taps, bk


def _chunks(total, step=512):
    out = []
    o = 0
    while o < total:
        out.append((o, min(step, total - o)))
        o += step
    return out


def _build_program(nkt):
    """One SPMD program; all cores run NKT key tiles, masks differ per core."""
    lkw = nkt * P
    nc = bacc.Bacc(get_trn_type() or "TRN2", target_bir_lowering=False, debug=False)
    evoT = nc.declare_dram_parameter("evoT", [Q_IN, L + 4], F32, isOutput=False)
    plmT = nc.declare_dram_parameter("plmT", [V_IN, L], F32, isOutput=False)
    wq = nc.declare_dram_parameter("wq", [Q_IN, QK], F32, isOutput=False)
    wk = nc.declare_dram_parameter("wk", [5 * Q_IN, QK], F32, isOutput=False)
    wv = nc.declare_dram_parameter("wv", [V_IN, QK], F32, isOutput=False)
    qb = nc.declare_dram_parameter("qb", [QK, 1], F32, isOutput=False)
    kb = nc.declare_dram_parameter("kb", [QK, 1], F32, isOutput=False)
    vb = nc.declare_dram_parameter("vb", [QK, 1], F32, isOutput=False)
    maskd = nc.declare_dram_parameter("mask", [P, nkt], F32, isOutput=False)
    identd = nc.declare_dram_parameter("ident", [P, P], F32, isOutput=False)
    ot_out = nc.declare_dram_parameter("ot", [QK + 1, L], F32, isOutput=True)
    vt_out = nc.declare_dram_parameter("vt", [QK, L], F32, isOutput=True)

    add = mybir.AluOpType.add

    with tile.TileContext(nc) as tc:
        with tc.tile_pool(name="sing", bufs=1) as sing:
            # ---- load inputs (split DMAs to spread across queues) ----
            evo_sb = []
            for i in range(4):
                t = sing.tile([P, L + 4], F32, tag=f"evo{i}")
                nc.sync.dma_start(out=t, in_=evoT[i * P : (i + 1) * P, :])
                evo_sb.append(t)
            plm_sb = []
            for i in range(8):
                t = sing.tile([P, L], F32, tag=f"plm{i}")
                nc.sync.dma_start(out=t[:, : L // 2], in_=plmT[i * P : (i + 1) * P, : L // 2])
                nc.sync.dma_start(out=t[:, L // 2 :], in_=plmT[i * P : (i + 1) * P, L // 2 :])
                plm_sb.append(t)
            wq_sb = sing.tile([P, 4, QK], F32, tag="wq")
            nc.sync.dma_start(out=wq_sb, in_=wq[:, :].rearrange("(n p) o -> p n o", p=P))
            wk_sb = sing.tile([P, 20, QK], F32, tag="wk")
            nc.sync.dma_start(out=wk_sb, in_=wk[:, :].rearrange("(n p) o -> p n o", p=P))
            wv_sb = sing.tile([P, 8, QK], F32, tag="wv")
            nc.sync.dma_start(out=wv_sb, in_=wv[:, :].rearrange("(n p) o -> p n o", p=P))
            qb_sb = sing.tile([QK, 1], F32, tag="qb")
            nc.sync.dma_start(out=qb_sb, in_=qb[:, :])
            kb_sb = sing.tile([QK, 1], F32, tag="kb")
            nc.sync.dma_start(out=kb_sb, in_=kb[:, :])
            vb_sb = sing.tile([QK, 1], F32, tag="vb")
            nc.sync.dma_start(out=vb_sb, in_=vb[:, :])
            mask_sb = sing.tile([P, nkt], F32, tag="mask")
            nc.sync.dma_start(out=mask_sb, in_=maskd[:, :])
            ident_sb = sing.tile([P, P], F32, tag="ident")
            nc.sync.dma_start(out=ident_sb, in_=identd[:, :])

            qt_sb = sing.tile([QK, L], F32, tag="qt")
            kt_sb = sing.tile([QK, lkw], F32, tag="kt")
            vt_sb = sing.tile([QK, L], F32, tag="vt")
            v1_sb = sing.tile([P, nkt, QK + 1], F32, tag="v1")
            ot_sb = sing.tile([QK + 1, L], F32, tag="ot")

            # ---- projections ----
            with (
                tc.tile_pool(name="proj_psum", bufs=3, space="PSUM") as proj_psum,
                tc.tile_pool(name="v1_psum", bufs=2, space="PSUM") as v1_psum,
            ):
                # QT = wq.T @ evoT  (+qb)
                for base, width in _chunks(L, 1024):
                    pt = proj_psum.tile([QK, 1024], F32, tag="proj")
                    for dt in range(4):
                        for o2, w2 in _chunks(width, 512):
                            nc.tensor.matmul(
                                pt[:, o2 : o2 + w2],
                                lhsT=wq_sb[:, dt, :],
                                rhs=evo_sb[dt][:, 2 + base + o2 : 2 + base + o2 + w2],
                                start=(dt == 0),
                                stop=(dt == 3),
                            )
                    nc.vector.tensor_scalar(
                        out=qt_sb[:, base : base + width],
                        in0=pt[:, :width],
                        scalar1=qb_sb,
                        scalar2=None,
                        op0=add,
                    )
                # KT = sum_t taps[t].T @ evoT(shift t-2)  (+kb), only first lkw cols
                for base, width in _chunks(lkw, 1024):
                    pt = proj_psum.tile([QK, 1024], F32, tag="proj")
                    n = 0
                    for t in range(5):
                        for dt in range(4):
                            for o2, w2 in _chunks(width, 512):
                                nc.tensor.matmul(
                                    pt[:, o2 : o2 + w2],
                                    lhsT=wk_sb[:, t * 4 + dt, :],
                                    rhs=evo_sb[dt][:, t + base + o2 : t + base + o2 + w2],
                                    start=(n == 0),
                                    stop=(n == 19),
                                )
                            n += 1
                    nc.vector.tensor_scalar(
                        out=kt_sb[:, base : base + width],
                        in0=pt[:, :width],
                        scalar1=kb_sb,
                        scalar2=None,
                        op0=add,
                    )
                # VT = wv.T @ plmT (+vb), full L (residual needs all of V)
                for base, width in _chunks(L, 1024):
                    pt = proj_psum.tile([QK, 1024], F32, tag="proj")
                    for dt in range(8):
                        for o2, w2 in _chunks(width, 512):
                            nc.tensor.matmul(
                                pt[:, o2 : o2 + w2],
                                lhsT=wv_sb[:, dt, :],
                                rhs=plm_sb[dt][:, base + o2 : base + o2 + w2],
                                start=(dt == 0),
                                stop=(dt == 7),
                            )
                    nc.vector.tensor_scalar(
                        out=vt_sb[:, base : base + width],
                        in0=pt[:, :width],
                        scalar1=vb_sb,
                        scalar2=None,
                        op0=add,
                    )
                nc.sync.dma_start(out=vt_out[:, :], in_=vt_sb)

                # V1[j] = [V natural | ones]  via PE transpose of VT slices
                for j in range(nkt):
                    vp = v1_psum.tile([P, QK], F32, tag="v1p")
                    nc.tensor.transpose(
                        vp, vt_sb[:, j * P : (j + 1) * P], ident_sb[:QK, :QK]
                    )
                    nc.vector.tensor_copy(out=v1_sb[:, j, :QK], in_=vp)
                    nc.vector.memset(v1_sb[:, j, QK : QK + 1], 1.0)

            # ---- attention (flash-style over l_q halves) ----
            with (
                tc.tile_pool(name="st_psum", bufs=2, space="PSUM") as st_psum,
                tc.tile_pool(name="ot_psum", bufs=2, space="PSUM") as ot_psum,
                tc.tile_pool(name="et", bufs=3) as et_pool,
            ):
                for half in range(2):
                    hb = half * (L // 2)
                    otp = ot_psum.tile([QK + 1, L // 2], F32, tag="otp")
                    for j in range(nkt):
                        stp = st_psum.tile([P, L // 2], F32, tag="stp")
                        for o2, w2 in _chunks(L // 2, 512):
                            nc.tensor.matmul(
                                stp[:, o2 : o2 + w2],
                                lhsT=kt_sb[:, j * P : (j + 1) * P],
                                rhs=qt_sb[:, hb + o2 : hb + o2 + w2],
                                start=True,
                                stop=True,
                            )
                        et = et_pool.tile([P, L // 2], F32, tag="et")
                        nc.scalar.activation(
                            out=et,
                            in_=stp,
                            func=mybir.ActivationFunctionType.Exp,
                            bias=mask_sb[:, j : j + 1],
                            scale=NORM,
                        )
                        for o2, w2 in _chunks(L // 2, 512):
                            nc.tensor.matmul(
                                otp[:, o2 : o2 + w2],
                                lhsT=v1_sb[:, j, :],
                                rhs=et[:, o2 : o2 + w2],
                                start=(j == 0),
                                stop=(j == nkt - 1),
                            )
                    nc.vector.tensor_copy(out=ot_sb[:, hb : hb + L // 2], in_=otp)
                nc.sync.dma_start(out=ot_out[:, :], in_=ot_sb)
    nc.finalize()
    return nc


def _prep_core_inputs(b, evo, plm, seqlen, weights, nkt):
    lkw = nkt * P
    evoT = np.zeros((Q_IN, L + 4), np.float32)
    evoT[:, 2 : 2 + L] = np.ascontiguousarray(evo.T)
    plmT = np.ascontiguousarray(plm.T)
    j = np.arange(nkt)[None, :]
    p = np.arange(P)[:, None]
    mask = np.where(j * P + p < seqlen, 0.0, -1e6).astype(np.float32)
    m = {
        "evoT": evoT,
        "plmT": plmT,
        "mask": mask,
    }
    m.update(weights)
    return m


def kernel(
    plm_embedding,
    evo_local,
    seqlengths,
    q_w,
    q_b,
    k_w,
    k_b,
    v_w,
    v_b,
    cn3_w,
    cn3_b,
    cn5_w,
    cn5_b,
):
    global LAST_EXEC_TIME_NS, LAST_RESULTS
    plm_embedding = np.asarray(plm_embedding, np.float32)
    evo_local = np.asarray(evo_local, np.float32)
    seqlengths = np.asarray(seqlengths)

    taps, bk = _fold_k_weights(
        np.asarray(k_w, np.float32),
        np.asarray(k_b, np.float32),
        np.asarray(cn3_w, np.float32),
        np.asarray(cn3_b, np.float32),
        np.asarray(cn5_w, np.float32),
        np.asarray(cn5_b, np.float32),
    )
    nkt = int(min(L // P, (int(seqlengths.max()) + P - 1) // P))
    weights = {
        "wq": np.ascontiguousarray(np.asarray(q_w, np.float32).T),
        "wk": np.ascontiguousarray(taps.reshape(5 * Q_IN, QK)),
        "wv": np.ascontiguousarray(np.asarray(v_w, np.float32).T),
        "qb": np.asarray(q_b, np.float32).reshape(QK, 1),
        "kb": bk.reshape(QK, 1),
        "vb": np.asarray(v_b, np.float32).reshape(QK, 1),
        "ident": np.eye(P, dtype=np.float32),
    }

    if nkt not in _program_cache:
        _program_cache[nkt] = _build_program(nkt)
    nc = _program_cache[nkt]

    in_maps = [
        _prep_core_inputs(
            b, evo_local[b], plm_embedding[b], int(seqlengths[b]), weights, nkt
        )
        for b in range(B)
    ]
    trace = bool(os.environ.get("KBENCH_TRACE"))
    res = run_bass_kernel_spmd(nc, in_maps, list(range(B)), trace=trace)
    LAST_EXEC_TIME_NS = res.exec_time_ns
    LAST_RESULTS = res

    out = np.empty((B, L, VD), np.float32)
    for b in range(B):
        ot = res.results[b]["ot"]
        vt = res.results[b]["vt"]
        out[b] = (ot[:QK] / ot[QK : QK + 1]).T + vt.T
    return out


# revision 7
# speedup vs baseline: 2.6489x; 2.6489x over previous
"""Contextual-attention Trainium2 kernel (Bass/Tile), data-parallel over batch.

Math (per sequence b):
    Q = evo @ q_w.T + q_b                                  (L, 96)
    K = cat(evo, conv3(evo), conv5(evo)) @ k_w.T + k_b     (L, 96)
    V = plm @ v_w.T + v_b                                  (L, 96)
    P = softmax(Q K^T / sqrt(96), key-masked by seqlen)
    out = P @ V + V

Device-side reformulation (per core = one sequence):
  * The two convs + concat + K-projection fold into 5 shifted matmuls:
        K[l] = sum_{t=-2..2} evo[l+t] @ taps[t]  + bk      (host-folded weights)
  * Everything is computed transposed ([feature, L] layout) so the only
    contraction layouts needed are natural ones:
        QT = wqT.T @ evoT, KT = taps.T @ evoT(shifted), VT = wvT.T @ plmT
        ST[lk, lq] = KT_slice.T @ QT  -> exp via ACT with per-partition mask bias
        OT[0:96]   = sum_lk V1[lk].T @ ET[lk]   (V1 = [V | ones] natural layout,
        OT[96]     = softmax denominator         via on-chip PE transpose of VT)
  * All matmul operands are fp16 (PE streams 2B/cycle: fp32 is half rate), all
    accumulation is f32 in PSUM; exp runs in f32 on ScalarE. fp16 (not bf16)
    because every tensor here is O(1)-ranged and fp16 carries 3 more mantissa
    bits.
  * Key tiles entirely beyond max(seqlen) are skipped at build time; the
    per-core mask bias (0 / -1e6) zeroes partially-valid tiles exactly
    (exp(-1e6 + s) underflows to 0.0f, matching the reference's where()+softmax).
  * Final divide by denominator, +V residual, and the (96, L) -> (L, 96)
    transpose happen on host (tiny O(L*96) work).
"""

import os
import numpy as np

import concourse.bacc as bacc
import concourse.bass as bass
import concourse.tile as tile
from concourse import mybir
from concourse._compat import get_trn_type
from concourse.bass_utils import run_bass_kernel_spmd

B, L = 8, 2048
Q_IN, V_IN, QK, VD = 512, 1024, 96, 96
P = 128
NORM = float(1.0 / np.sqrt(QK))
F32 = mybir.dt.float32
F16 = mybir.dt.float16

LAST_EXEC_TIME_NS = None
LAST_RESULTS = None

_program_cache = {}


def _fold_k_weights(k_w, k_b, cn3_w, cn3_b, cn5_w, cn5_b):
    """K[l] = sum_{t in -2..2} evo[l+t] @ taps[t+2] + bk  (zero-padded shifts)."""
    A_evo = k_w[:, :Q_IN]
    A3 = k_w[:, Q_IN : Q_IN + VD]
    A5 = k_w[:, Q_IN + VD :]
    taps = np.zeros((5, Q_IN, QK), np.float32)
    for j in range(3):  # conv3 tap j acts at offset t = j-1
        taps[j - 1 + 2] += np.einsum("oc,cd->do", A3, cn3_w[:, :, j]).astype(np.float32)
    for j in range(5):  # conv5 tap j acts at offset t = j-2
        taps[j - 2 + 2] += np.einsum("oc,cd->do", A5, cn5_w[:, :, j]).astype(np.float32)
    taps[2] += A_evo.T
    bk = (k_b + A3 @ cn3_b + A5 @ cn5_b).astype(np.float32)
    return taps, bk


def _chunks(total, step=512):
    out = []
    o = 0
    while o < total:
        out.append((o, min(step, total - o)))
        o += step
    return out


def _build_program(nkt):
    """One SPMD program; all cores run NKT key tiles, masks differ per core."""
    lkw = nkt * P
    nc = bacc.Bacc(get_trn_type() or "TRN2", target_bir_lowering=False, debug=False)
    # weight/constant params (tiny, loaded first)
    wq = nc.declare_dram_parameter("wq", [P, 4 * QK], F16, isOutput=False)
    wk = nc.declare_dram_parameter("wk", [P, 20 * QK], F16, isOutput=False)
    wv = nc.declare_dram_parameter("wv", [P, 8 * QK], F16, isOutput=False)
    bqkv = nc.declare_dram_parameter("bqkv", [QK, 3], F32, isOutput=False)
    maskd = nc.declare_dram_parameter("mask", [P, nkt], F32, isOutput=False)
    identd = nc.declare_dram_parameter("ident", [P, P], F16, isOutput=False)
    # activations
    evoT = nc.declare_dram_parameter("evoT", [Q_IN, L + 4], F16, isOutput=False)
    plmT = nc.declare_dram_parameter("plmT", [V_IN, L], F16, isOutput=False)
    # outputs
    ot_out = nc.declare_dram_parameter("ot", [QK + 1, L], F32, isOutput=True)
    vt_out = nc.declare_dram_parameter("vt", [QK, L], F32, isOutput=True)

    add = mybir.AluOpType.add

    with tile.TileContext(nc) as tc:
        with tc.tile_pool(name="sing", bufs=1) as sing:
            # ---- constants & weights first (they gate the first matmuls) ----
            wq_sb = sing.tile([P, 4, QK], F16, tag="wq")
            nc.sync.dma_start(out=wq_sb, in_=wq[:, :].rearrange("p (n o) -> p n o", o=QK))
            wk_sb = sing.tile([P, 20, QK], F16, tag="wk")
            nc.sync.dma_start(out=wk_sb, in_=wk[:, :].rearrange("p (n o) -> p n o", o=QK))
            wv_sb = sing.tile([P, 8, QK], F16, tag="wv")
            nc.sync.dma_start(out=wv_sb, in_=wv[:, :].rearrange("p (n o) -> p n o", o=QK))
            b_sb = sing.tile([QK, 3], F32, tag="bqkv")
            nc.sync.dma_start(out=b_sb, in_=bqkv[:, :])
            mask_sb = sing.tile([P, nkt], F32, tag="mask")
            nc.sync.dma_start(out=mask_sb, in_=maskd[:, :])
            ident_sb = sing.tile([P, P], F16, tag="ident")
            nc.sync.dma_start(out=ident_sb, in_=identd[:, :])

            # ---- activations (evo first: unblocks Q/K projections) ----
            evo_sb = []
            for i in range(4):
                t = sing.tile([P, L + 4], F16, tag=f"evo{i}")
                nc.sync.dma_start(out=t, in_=evoT[i * P : (i + 1) * P, :])
                evo_sb.append(t)
            plm_sb = []
            for i in range(8):
                t = sing.tile([P, L], F16, tag=f"plm{i}")
                nc.sync.dma_start(out=t, in_=plmT[i * P : (i + 1) * P, :])
                plm_sb.append(t)

            qt_sb = sing.tile([QK, L], F16, tag="qt")
            kt_sb = sing.tile([QK, lkw], F16, tag="kt")
            vt_sb = sing.tile([QK, L], F32, tag="vt")
            vt16_sb = sing.tile([QK, L], F16, tag="vt16")
            v1_sb = sing.tile([P, nkt, QK + 1], F16, tag="v1")
            ot_sb = sing.tile([QK + 1, L], F32, tag="ot")

            # ---- projections ----
            with (
                tc.tile_pool(name="proj_psum", bufs=3, space="PSUM") as proj_psum,
                tc.tile_pool(name="v1_psum", bufs=2, space="PSUM") as v1_psum,
            ):
                # QT = wq.T @ evoT  (+qb)
                for base, width in _chunks(L, 1024):
                    pt = proj_psum.tile([QK, 1024], F32, tag="proj")
                    for dt in range(4):
                        for o2, w2 in _chunks(width, 512):
                            nc.tensor.matmul(
                                pt[:, o2 : o2 + w2],
                                lhsT=wq_sb[:, dt, :],
                                rhs=evo_sb[dt][:, 2 + base + o2 : 2 + base + o2 + w2],
                                start=(dt == 0),
                                stop=(dt == 3),
                            )
                    nc.vector.tensor_scalar(
                        out=qt_sb[:, base : base + width],
                        in0=pt[:, :width],
                        scalar1=b_sb[:, 0:1],
                        scalar2=None,
                        op0=add,
                    )
                # KT = sum_t taps[t].T @ evoT(shift t-2)  (+kb), first lkw cols only
                for base, width in _chunks(lkw, 1024):
                    pt = proj_psum.tile([QK, 1024], F32, tag="proj")
                    n = 0
                    for t in range(5):
                        for dt in range(4):
                            for o2, w2 in _chunks(width, 512):
                                nc.tensor.matmul(
                                    pt[:, o2 : o2 + w2],
                                    lhsT=wk_sb[:, t * 4 + dt, :],
                                    rhs=evo_sb[dt][:, t + base + o2 : t + base + o2 + w2],
                                    start=(n == 0),
                                    stop=(n == 19),
                                )
                            n += 1
                    nc.vector.tensor_scalar(
                        out=kt_sb[:, base : base + width],
                        in0=pt[:, :width],
                        scalar1=b_sb[:, 1:2],
                        scalar2=None,
                        op0=add,
                    )
                # VT = wv.T @ plmT (+vb), full L (residual needs all of V)
                for base, width in _chunks(L, 1024):
                    pt = proj_psum.tile([QK, 1024], F32, tag="proj")
                    for dt in range(8):
                        for o2, w2 in _chunks(width, 512):
                            nc.tensor.matmul(
                                pt[:, o2 : o2 + w2],
                                lhsT=wv_sb[:, dt, :],
                                rhs=plm_sb[dt][:, base + o2 : base + o2 + w2],
                                start=(dt == 0),
                                stop=(dt == 7),
                            )
                    nc.vector.tensor_scalar(
                        out=vt_sb[:, base : base + width],
                        in0=pt[:, :width],
                        scalar1=b_sb[:, 2:3],
                        scalar2=None,
                        op0=add,
                    )
                    nc.scalar.copy(
                        out=vt16_sb[:, base : base + width],
                        in_=vt_sb[:, base : base + width],
                    )
                    nc.sync.dma_start(
                        out=vt_out[:, base : base + width],
                        in_=vt_sb[:, base : base + width],
                    )

                # V1[j] = [V natural | ones]  via PE transpose of VT slices
                for j in range(nkt):
                    vp = v1_psum.tile([P, QK], F16, tag="v1p")
                    nc.tensor.transpose(
                        vp, vt16_sb[:, j * P : (j + 1) * P], ident_sb[:QK, :QK]
                    )
                    nc.vector.tensor_copy(out=v1_sb[:, j, :QK], in_=vp)
                    nc.vector.memset(v1_sb[:, j, QK : QK + 1], 1.0)

            # ---- attention (flash-style over l_q halves) ----
            with (
                tc.tile_pool(name="st_psum", bufs=2, space="PSUM") as st_psum,
                tc.tile_pool(name="ot_psum", bufs=2, space="PSUM") as ot_psum,
                tc.tile_pool(name="et", bufs=nkt + 2) as et_pool,
            ):
                for half in range(2):
                    hb = half * (L // 2)
                    ets = []
                    # scores + exp for the whole half (independent of V/plm)
                    for j in range(nkt):
                        stp = st_psum.tile([P, L // 2], F32, tag="stp")
                        for o2, w2 in _chunks(L // 2, 512):
                            nc.tensor.matmul(
                                stp[:, o2 : o2 + w2],
                                lhsT=kt_sb[:, j * P : (j + 1) * P],
                                rhs=qt_sb[:, hb + o2 : hb + o2 + w2],
                                start=True,
                                stop=True,
                            )
                        et = et_pool.tile([P, L // 2], F16, tag="et")
                        nc.scalar.activation(
                            out=et,
                            in_=stp,
                            func=mybir.ActivationFunctionType.Exp,
                            bias=mask_sb[:, j : j + 1],
                            scale=NORM,
                        )
                        ets.append(et)
                    # O^T accumulation (needs V1, i.e. plm)
                    otp = ot_psum.tile([QK + 1, L // 2], F32, tag="otp")
                    for j in range(nkt):
                        for o2, w2 in _chunks(L // 2, 512):
                            nc.tensor.matmul(
                                otp[:, o2 : o2 + w2],
                                lhsT=v1_sb[:, j, :],
                                rhs=ets[j][:, o2 : o2 + w2],
                                start=(j == 0),
                                stop=(j == nkt - 1),
                            )
                    nc.vector.tensor_copy(out=ot_sb[:, hb : hb + L // 2], in_=otp)
                    nc.sync.dma_start(
                        out=ot_out[:, hb : hb + L // 2], in_=ot_sb[:, hb : hb + L // 2]
                    )
    nc.finalize()
    return nc


def _prep_core_inputs(evo, plm, seqlen, weights, nkt):
    evoT = np.zeros((Q_IN, L + 4), np.float16)
    evoT[:, 2 : 2 + L] = evo.T
    plmT = np.ascontiguousarray(plm.T.astype(np.float16))
    j = np.arange(nkt)[None, :]
    p = np.arange(P)[:, None]
    mask = np.where(j * P + p < seqlen, 0.0, -1e6).astype(np.float32)
    m = {"evoT": evoT, "plmT": plmT, "mask": mask}
    m.update(weights)
    return m


def _pack_w(w, n):
    # (n*128, 96) f32 -> (128, n*96) f16 in the SBUF [p, n, o] layout
    return np.ascontiguousarray(
        w.reshape(n, P, QK).transpose(1, 0, 2).reshape(P, n * QK).astype(np.float16)
    )


def kernel(
    plm_embedding,
    evo_local,
    seqlengths,
    q_w,
    q_b,
    k_w,
    k_b,
    v_w,
    v_b,
    cn3_w,
    cn3_b,
    cn5_w,
    cn5_b,
):
    global LAST_EXEC_TIME_NS, LAST_RESULTS
    plm_embedding = np.asarray(plm_embedding, np.float32)
    evo_local = np.asarray(evo_local, np.float32)
    seqlengths = np.asarray(seqlengths)

    taps, bk = _fold_k_weights(
        np.asarray(k_w, np.float32),
        np.asarray(k_b, np.float32),
        np.asarray(cn3_w, np.float32),
        np.asarray(cn3_b, np.float32),
        np.asarray(cn5_w, np.float32),
        np.asarray(cn5_b, np.float32),
    )
    nkt = int(min(L // P, (int(seqlengths.max()) + P - 1) // P))
    bqkv = np.stack(
        [np.asarray(q_b, np.float32), bk, np.asarray(v_b, np.float32)], axis=1
    ).astype(np.float32)
    weights = {
        "wq": _pack_w(np.ascontiguousarray(np.asarray(q_w, np.float32).T), 4),
        "wk": _pack_w(taps.reshape(5 * Q_IN, QK), 20),
        "wv": _pack_w(np.ascontiguousarray(np.asarray(v_w, np.float32).T), 8),
        "bqkv": np.ascontiguousarray(bqkv),
        "ident": np.eye(P, dtype=np.float16),
    }

    if nkt not in _program_cache:
        _program_cache[nkt] = _build_program(nkt)
    nc = _program_cache[nkt]

    in_maps = [
        _prep_core_inputs(evo_local[b], plm_embedding[b], int(seqlengths[b]), weights, nkt)
        for b in range(B)
    ]
    trace = bool(os.environ.get("KBENCH_TRACE"))
    res = run_bass_kernel_spmd(nc, in_maps, list(range(B)), trace=trace)
    LAST_EXEC_TIME_NS = res.exec_time_ns
    LAST_RESULTS = res

    out = np.empty((B, L, VD), np.float32)
    for b in range(B):
        ot = res.results[b]["ot"]
        vt = res.results[b]["vt"]
        out[b] = (ot[:QK] / ot[QK : QK + 1]).T + vt.T
    return out


# revision 11
# speedup vs baseline: 3.2574x; 1.2297x over previous
"""Contextual-attention Trainium2 kernel (Bass/Tile), data-parallel over batch.

Math (per sequence b):
    Q = evo @ q_w.T + q_b                                  (L, 96)
    K = cat(evo, conv3(evo), conv5(evo)) @ k_w.T + k_b     (L, 96)
    V = plm @ v_w.T + v_b                                  (L, 96)
    P = softmax(Q K^T / sqrt(96), key-masked by seqlen)
    out = P @ V + V

Device-side reformulation (per core = one sequence):
  * The two convs + concat + K-projection fold into 5 shifted matmuls:
        K[l] = sum_{t=-2..2} evo[l+t] @ taps[t]  + bk      (host-folded weights)
  * Everything is computed transposed ([feature, L] layout) so the only
    contraction layouts needed are natural ones:
        QT = wqT.T @ evoT, KT = taps.T @ evoT(shifted), VT = wvT.T @ plmT
        ST[lk, lq] = KT_slice.T @ QT  -> exp via ACT with per-partition mask bias
        OT[0:96]   = sum_lk V1[lk].T @ ET[lk]   (V1 = [V | ones] natural layout,
        OT[96]     = softmax denominator         via on-chip PE transpose of VT)
  * All matmul operands are fp16 (PE streams 2B/cycle: fp32 is half rate), all
    accumulation is f32 in PSUM; exp runs in f32 on ScalarE. fp16 (not bf16)
    because every tensor here is O(1)-ranged and fp16 carries 3 more mantissa
    bits.
  * Key tiles entirely beyond max(seqlen) are skipped at build time; the
    per-core mask bias (0 / -1e6) zeroes partially-valid tiles exactly
    (exp(-1e6 + s) underflows to 0.0f, matching the reference's where()+softmax).
  * Final divide by denominator, +V residual, and the (96, L) -> (L, 96)
    transpose happen on host (tiny O(L*96) work).
"""

import os
import numpy as np

import concourse.bacc as bacc
import concourse.bass as bass
import concourse.tile as tile
from concourse import mybir
from concourse._compat import get_trn_type
from concourse.bass_utils import run_bass_kernel_spmd

B, L = 8, 2048
Q_IN, V_IN, QK, VD = 512, 1024, 96, 96
P = 128
NORM = float(1.0 / np.sqrt(QK))
F32 = mybir.dt.float32
F16 = mybir.dt.float16

LAST_EXEC_TIME_NS = None
LAST_RESULTS = None

_program_cache = {}


def _fold_k_weights(k_w, k_b, cn3_w, cn3_b, cn5_w, cn5_b):
    """K[l] = sum_{t in -2..2} evo[l+t] @ taps[t+2] + bk  (zero-padded shifts)."""
    A_evo = k_w[:, :Q_IN]
    A3 = k_w[:, Q_IN : Q_IN + VD]
    A5 = k_w[:, Q_IN + VD :]
    taps = np.zeros((5, Q_IN, QK), np.float32)
    for j in range(3):  # conv3 tap j acts at offset t = j-1
        taps[j - 1 + 2] += np.einsum("oc,cd->do", A3, cn3_w[:, :, j]).astype(np.float32)
    for j in range(5):  # conv5 tap j acts at offset t = j-2
        taps[j - 2 + 2] += np.einsum("oc,cd->do", A5, cn5_w[:, :, j]).astype(np.float32)
    taps[2] += A_evo.T
    bk = (k_b + A3 @ cn3_b + A5 @ cn5_b).astype(np.float32)
    return taps, bk


def _chunks(total, step=512):
    out = []
    o = 0
    while o < total:
        out.append((o, min(step, total - o)))
        o += step
    return out


def _build_program(nkt):
    """One SPMD program; all cores run NKT key tiles, masks differ per core."""
    lkw = nkt * P
    nc = bacc.Bacc(get_trn_type() or "TRN2", target_bir_lowering=False, debug=False)
    # weight/constant params (tiny, loaded first)
    wq = nc.declare_dram_parameter("wq", [P, 4 * QK], F16, isOutput=False)
    wk = nc.declare_dram_parameter("wk", [P, 20 * QK], F16, isOutput=False)
    wv = nc.declare_dram_parameter("wv", [P, 8 * QK], F16, isOutput=False)
    bqkv = nc.declare_dram_parameter("bqkv", [QK, 3], F32, isOutput=False)
    maskd = nc.declare_dram_parameter("mask", [P, nkt], F32, isOutput=False)
    identd = nc.declare_dram_parameter("ident", [P, P], F16, isOutput=False)
    # activations
    evoT = nc.declare_dram_parameter("evoT", [Q_IN, L + 4], F16, isOutput=False)
    plmT = nc.declare_dram_parameter("plmT", [V_IN, L], F16, isOutput=False)
    # outputs
    ot_out = nc.declare_dram_parameter("ot", [QK + 1, L], F32, isOutput=True)
    vt_out = nc.declare_dram_parameter("vt", [QK, L], F16, isOutput=True)

    add = mybir.AluOpType.add

    with tile.TileContext(nc) as tc:
        with tc.tile_pool(name="sing", bufs=1) as sing:
            # ---- constants & weights first (they gate the first matmuls) ----
            wq_sb = sing.tile([P, 4, QK], F16, tag="wq")
            nc.sync.dma_start(out=wq_sb, in_=wq[:, :].rearrange("p (n o) -> p n o", o=QK))
            wk_sb = sing.tile([P, 20, QK], F16, tag="wk")
            nc.sync.dma_start(out=wk_sb, in_=wk[:, :].rearrange("p (n o) -> p n o", o=QK))
            wv_sb = sing.tile([P, 8, QK], F16, tag="wv")
            nc.sync.dma_start(out=wv_sb, in_=wv[:, :].rearrange("p (n o) -> p n o", o=QK))
            b_sb = sing.tile([QK, 3], F32, tag="bqkv")
            nc.sync.dma_start(out=b_sb, in_=bqkv[:, :])
            mask_sb = sing.tile([P, nkt], F32, tag="mask")
            nc.sync.dma_start(out=mask_sb, in_=maskd[:, :])
            ident_sb = sing.tile([P, P], F16, tag="ident")
            nc.sync.dma_start(out=ident_sb, in_=identd[:, :])

            # ---- activations (evo first: unblocks Q/K projections) ----
            evo_sb = []
            for i in range(4):
                t = sing.tile([P, L + 4], F16, tag=f"evo{i}")
                for h in range(2):  # partition-halves -> 2 DMA engines per tile
                    nc.sync.dma_start(
                        out=t[h * 64 : (h + 1) * 64, :],
                        in_=evoT[i * P + h * 64 : i * P + (h + 1) * 64, :],
                    )
                evo_sb.append(t)
            plm_sb = []
            for i in range(8):
                t = sing.tile([P, L], F16, tag=f"plm{i}")
                for h in range(2):
                    nc.sync.dma_start(
                        out=t[h * 64 : (h + 1) * 64, :],
                        in_=plmT[i * P + h * 64 : i * P + (h + 1) * 64, :],
                    )
                plm_sb.append(t)

            qt_sb = sing.tile([QK, L], F16, tag="qt")
            kt_sb = sing.tile([QK, lkw], F16, tag="kt")
            vt_sb = sing.tile([QK, L], F32, tag="vt")
            vt16_sb = sing.tile([QK, L], F16, tag="vt16")
            v1_sb = sing.tile([P, nkt, QK + 1], F16, tag="v1")
            ot_sb = sing.tile([QK + 1, L], F32, tag="ot")

            # ---- projections ----
            with (
                tc.tile_pool(name="proj_psum", bufs=3, space="PSUM") as proj_psum,
                tc.tile_pool(name="v1_psum", bufs=2, space="PSUM") as v1_psum,
            ):
                # QT = wq.T @ evoT  (+qb)
                for base, width in _chunks(L, 1024):
                    pt = proj_psum.tile([QK, 1024], F32, tag="proj")
                    for dt in range(4):
                        for o2, w2 in _chunks(width, 512):
                            nc.tensor.matmul(
                                pt[:, o2 : o2 + w2],
                                lhsT=wq_sb[:, dt, :],
                                rhs=evo_sb[dt][:, 2 + base + o2 : 2 + base + o2 + w2],
                                start=(dt == 0),
                                stop=(dt == 3),
                            )
                    nc.vector.tensor_scalar(
                        out=qt_sb[:, base : base + width],
                        in0=pt[:, :width],
                        scalar1=b_sb[:, 0:1],
                        scalar2=None,
                        op0=add,
                    )
                # KT = sum_t taps[t].T @ evoT(shift t-2)  (+kb), first lkw cols only
                for base, width in _chunks(lkw, 1024):
                    pt = proj_psum.tile([QK, 1024], F32, tag="proj")
                    n = 0
                    for t in range(5):
                        for dt in range(4):
                            for o2, w2 in _chunks(width, 512):
                                nc.tensor.matmul(
                                    pt[:, o2 : o2 + w2],
                                    lhsT=wk_sb[:, t * 4 + dt, :],
                                    rhs=evo_sb[dt][:, t + base + o2 : t + base + o2 + w2],
                                    start=(n == 0),
                                    stop=(n == 19),
                                )
                            n += 1
                    nc.vector.tensor_scalar(
                        out=kt_sb[:, base : base + width],
                        in0=pt[:, :width],
                        scalar1=b_sb[:, 1:2],
                        scalar2=None,
                        op0=add,
                    )
                # VT = wv.T @ plmT (+vb), full L (residual needs all of V)
                for base, width in _chunks(L, 1024):
                    pt = proj_psum.tile([QK, 1024], F32, tag="proj")
                    for dt in range(8):
                        for o2, w2 in _chunks(width, 512):
                            nc.tensor.matmul(
                                pt[:, o2 : o2 + w2],
                                lhsT=wv_sb[:, dt, :],
                                rhs=plm_sb[dt][:, base + o2 : base + o2 + w2],
                                start=(dt == 0),
                                stop=(dt == 7),
                            )
                    nc.vector.tensor_scalar(
                        out=vt_sb[:, base : base + width],
                        in0=pt[:, :width],
                        scalar1=b_sb[:, 2:3],
                        scalar2=None,
                        op0=add,
                    )
                    nc.scalar.copy(
                        out=vt16_sb[:, base : base + width],
                        in_=vt_sb[:, base : base + width],
                    )
                    nc.sync.dma_start(
                        out=vt_out[:, base : base + width],
                        in_=vt16_sb[:, base : base + width],
                    )

                # V1[j] = [V natural | ones]  via PE transpose of VT slices
                for j in range(nkt):
                    vp = v1_psum.tile([P, QK], F16, tag="v1p")
                    nc.tensor.transpose(
                        vp, vt16_sb[:, j * P : (j + 1) * P], ident_sb[:QK, :QK]
                    )
                    nc.vector.tensor_copy(out=v1_sb[:, j, :QK], in_=vp)
                    nc.vector.memset(v1_sb[:, j, QK : QK + 1], 1.0)

            # ---- attention (flash-style over l_q halves) ----
            with (
                tc.tile_pool(name="st_psum", bufs=2, space="PSUM") as st_psum,
                tc.tile_pool(name="ot_psum", bufs=2, space="PSUM") as ot_psum,
                tc.tile_pool(name="et", bufs=nkt + 2) as et_pool,
            ):
                for half in range(2):
                    hb = half * (L // 2)
                    ets = []
                    # scores + exp for the whole half (independent of V/plm)
                    for j in range(nkt):
                        stp = st_psum.tile([P, L // 2], F32, tag="stp")
                        for o2, w2 in _chunks(L // 2, 512):
                            nc.tensor.matmul(
                                stp[:, o2 : o2 + w2],
                                lhsT=kt_sb[:, j * P : (j + 1) * P],
                                rhs=qt_sb[:, hb + o2 : hb + o2 + w2],
                                start=True,
                                stop=True,
                            )
                        et = et_pool.tile([P, L // 2], F16, tag="et")
                        nc.scalar.activation(
                            out=et,
                            in_=stp,
                            func=mybir.ActivationFunctionType.Exp,
                            bias=mask_sb[:, j : j + 1],
                            scale=NORM,
                        )
                        ets.append(et)
                    # O^T accumulation (needs V1, i.e. plm)
                    otp = ot_psum.tile([QK + 1, L // 2], F32, tag="otp")
                    for j in range(nkt):
                        for o2, w2 in _chunks(L // 2, 512):
                            nc.tensor.matmul(
                                otp[:, o2 : o2 + w2],
                                lhsT=v1_sb[:, j, :],
                                rhs=ets[j][:, o2 : o2 + w2],
                                start=(j == 0),
                                stop=(j == nkt - 1),
                            )
                    nc.vector.tensor_copy(out=ot_sb[:, hb : hb + L // 2], in_=otp)
                    # 97-partition DMAs defeat the DMA-engine fanout (must be a
                    # multiple of 16): store rows 0..95 and the denom row apart.
                    nc.sync.dma_start(
                        out=ot_out[:QK, hb : hb + L // 2],
                        in_=ot_sb[:QK, hb : hb + L // 2],
                    )
                    nc.scalar.dma_start(
                        out=ot_out[QK : QK + 1, hb : hb + L // 2],
                        in_=ot_sb[QK : QK + 1, hb : hb + L // 2],
                    )
    nc.finalize()
    return nc


def _prep_core_inputs(evo, plm, seqlen, weights, nkt):
    evoT = np.zeros((Q_IN, L + 4), np.float16)
    evoT[:, 2 : 2 + L] = evo.T
    plmT = np.ascontiguousarray(plm.T.astype(np.float16))
    j = np.arange(nkt)[None, :]
    p = np.arange(P)[:, None]
    mask = np.where(j * P + p < seqlen, 0.0, -1e6).astype(np.float32)
    m = {"evoT": evoT, "plmT": plmT, "mask": mask}
    m.update(weights)
    return m


def _pack_w(w, n):
    # (n*128, 96) f32 -> (128, n*96) f16 in the SBUF [p, n, o] layout
    return np.ascontiguousarray(
        w.reshape(n, P, QK).transpose(1, 0, 2).reshape(P, n * QK).astype(np.float16)
    )


def kernel(
    plm_embedding,
    evo_local,
    seqlengths,
    q_w,
    q_b,
    k_w,
    k_b,
    v_w,
    v_b,
    cn3_w,
    cn3_b,
    cn5_w,
    cn5_b,
):
    global LAST_EXEC_TIME_NS, LAST_RESULTS
    plm_embedding = np.asarray(plm_embedding, np.float32)
    evo_local = np.asarray(evo_local, np.float32)
    seqlengths = np.asarray(seqlengths)

    taps, bk = _fold_k_weights(
        np.asarray(k_w, np.float32),
        np.asarray(k_b, np.float32),
        np.asarray(cn3_w, np.float32),
        np.asarray(cn3_b, np.float32),
        np.asarray(cn5_w, np.float32),
        np.asarray(cn5_b, np.float32),
    )
    nkt = int(min(L // P, (int(seqlengths.max()) + P - 1) // P))
    bqkv = np.stack(
        [np.asarray(q_b, np.float32), bk, np.asarray(v_b, np.float32)], axis=1
    ).astype(np.float32)
    weights = {
        "wq": _pack_w(np.ascontiguousarray(np.asarray(q_w, np.float32).T), 4),
        "wk": _pack_w(taps.reshape(5 * Q_IN, QK), 20),
        "wv": _pack_w(np.ascontiguousarray(np.asarray(v_w, np.float32).T), 8),
        "bqkv": np.ascontiguousarray(bqkv),
        "ident": np.eye(P, dtype=np.float16),
    }

    if nkt not in _program_cache:
        _program_cache[nkt] = _build_program(nkt)
    nc = _program_cache[nkt]

    in_maps = [
        _prep_core_inputs(evo_local[b], plm_embedding[b], int(seqlengths[b]), weights, nkt)
        for b in range(B)
    ]
    trace = bool(os.environ.get("KBENCH_TRACE"))
    res = run_bass_kernel_spmd(nc, in_maps, list(range(B)), trace=trace)
    LAST_EXEC_TIME_NS = res.exec_time_ns
    LAST_RESULTS = res

    out = np.empty((B, L, VD), np.float32)
    for b in range(B):
        ot = res.results[b]["ot"]
        vt = res.results[b]["vt"]
        out[b] = (ot[:QK] / ot[QK : QK + 1]).T + vt.T
    return out
